# revision 1
# baseline (speedup 1.0000x reference)
"""CRF negative-log-likelihood kernel for Trainium2 (Bass/Tile), 8-core SPMD.

Problem: emission [128, 512, 32] f32, length [128], target [128, 512],
transition [32, 32], start/end_transition [32] -> scalar f32
  sum_b (log_partition_b - log_score_b)

Strategy (data-parallel over batch, 16 sequences per core):
  * log_partition via the forward algorithm run in EXP space so each step is
    one real matmul on TensorE:  A_{t} = (W^T A_{t-1}) .* E_t
    with per-(t,b) pre-normalization E_t = exp(em_t) / sum_j exp(em_t)
    (the log of the dropped scale, c_{t,b} = log sum_j exp(em[t,b,:]),
    is accumulated separately and added back at the end).
  * Variable lengths use an absorbing extra tag "omega" (index 32):
    W[i, omega] = exp(end_i), W[omega, omega] = 1, and the per-step
    multiplier for omega is 1 on padded steps / 0 on real steps.  All mass
    transitions into omega exactly at t = length_b, carrying the
    end_transition weight; full-length sequences never enter omega and get
    end_transition applied in the final reduction instead.
  * Scan state is [33, 16]: real tags at partition offset 0, omega at
    offset 32 (both 32-aligned).  The per-step E tiles come from PE
    transposes of the normalized exp-emission slab (4 timesteps per
    [16,128] chunk); omega multipliers live in a [1, 512*16] row.
  * log_score needs only its SUM over the batch, so it is computed with
    one-hot / count-matrix contractions (no gathers): emission term via a
    masked one-hot multiply in a [128, 64] full-partition relayout of
    (b, t); transition term via C[i,j] = #(valid t: tgt_t=i, tgt_{t+1}=j)
    built from 64 PSUM-accumulated matmuls, dotted with raw T; start/end
    terms via tiny one-hot count matmuls.
  * Each core writes one partial sum; the host adds the 8 partials.
"""

import numpy as np

B = 16           # batch per core
S = 512          # sequence length
J = 32           # tags
JA = J + 1       # augmented with omega
NCORES = 8
CHUNK_T = 4      # timesteps per PE transpose chunk (4*32 = 128)
NCHUNK = S // CHUNK_T
P = 128          # full partition count for the score relayout
FS = B * S // P  # 64 free elems per partition in the score relayout


def build_bass(scan_steps=S, with_score=True):
    import concourse.bacc as bacc
    import concourse.tile as tile
    from concourse import mybir

    f32 = mybir.dt.float32
    i32 = mybir.dt.int32

    nc = bacc.Bacc(
        "TRN2", target_bir_lowering=False, debug=False, num_devices=NCORES
    )

    em_d = nc.dram_tensor("emission", [B, S, J], f32, kind="ExternalInput")
    len_d = nc.dram_tensor("length", [B, 1], i32, kind="ExternalInput")
    tgt_d = nc.dram_tensor("target", [B, S], i32, kind="ExternalInput")
    T_d = nc.dram_tensor("transition", [J, J], f32, kind="ExternalInput")
    st_d = nc.dram_tensor("start_transition", [J, 1], f32, kind="ExternalInput")
    en_d = nc.dram_tensor("end_transition", [J, 1], f32, kind="ExternalInput")
    om_d = nc.dram_tensor("omega", [1, S * B], f32, kind="ExternalInput")
    out_d = nc.dram_tensor("out", [1, 1], f32, kind="ExternalOutput")

    Exp = mybir.ActivationFunctionType.Exp
    Ln = mybir.ActivationFunctionType.Ln
    Alu = mybir.AluOpType
    Ax = mybir.AxisListType

    with tile.TileContext(nc) as tc:
        with (
            tc.tile_pool(name="big", bufs=1) as big,        # persistent slabs
            tc.tile_pool(name="small", bufs=1) as small,    # persistent small
            tc.tile_pool(name="apool", bufs=3) as apool,    # scan state
            tc.tile_pool(name="pscan", bufs=2, space="PSUM") as pscan,
            tc.tile_pool(name="ptrans", bufs=2, space="PSUM") as ptrans,
            tc.tile_pool(name="pfin", bufs=3, space="PSUM") as pfin,
        ):
            # ---------------- load inputs ----------------
            len_i = small.tile([B, 1], i32, tag="len_i")
            nc.sync.dma_start(len_i[:], len_d.ap())

            # ---------------- masks (b-partition layout) ----------------
            tvec = small.tile([B, S], i32, tag="tvec")
            nc.gpsimd.iota(tvec[:], pattern=[[1, S]], base=0,
                           channel_multiplier=0)
            len_f = small.tile([B, 1], f32, tag="len_f")
            nc.vector.tensor_copy(len_f[:], len_i[:])
            tvec_f = small.tile([B, S], f32, tag="tvec_f")
            nc.vector.tensor_copy(tvec_f[:], tvec[:])
            mask = small.tile([B, S], f32, tag="mask")
            nc.vector.tensor_scalar(
                mask[:], tvec_f[:], len_f[:], None, op0=Alu.is_lt
            )

            # ---------------- prep slab preT[b, t, j] ----------------
            preT = big.tile([B, S * J], f32, tag="preT")
            preT3 = preT[:].rearrange("b (s j) -> b s j", j=J)
            s_sum = small.tile([B, S], f32, tag="s_sum")
            TCK = 128
            for ck in range(S // TCK):
                sl = slice(ck * TCK, (ck + 1) * TCK)
                nc.sync.dma_start(preT3[:, sl, :], em_d.ap()[:, sl, :])
                nc.scalar.activation(preT3[:, sl, :], preT3[:, sl, :], Exp)
                nc.vector.tensor_reduce(
                    s_sum[:, sl], preT3[:, sl, :], axis=Ax.X, op=Alu.add
                )
            # s_eff = (s_sum - 1) * mask + 1   (=1 on padded steps)
            s_eff = small.tile([B, S], f32, tag="s_eff")
            nc.vector.tensor_scalar(s_eff[:], s_sum[:], -1.0, None, op0=Alu.add)
            nc.vector.tensor_mul(s_eff[:], s_eff[:], mask[:])
            nc.vector.tensor_scalar(s_eff[:], s_eff[:], 1.0, None, op0=Alu.add)
            # rs_mask = mask / s_eff ; c_log = ln(s_eff) ; csum = sum_t c_log
            rs_mask = small.tile([B, S], f32, tag="rs_mask")
            nc.vector.reciprocal(rs_mask[:], s_eff[:])
            nc.vector.tensor_mul(rs_mask[:], rs_mask[:], mask[:])
            c_log = small.tile([B, S], f32, tag="c_log")
            nc.scalar.activation(c_log[:], s_eff[:], Ln)
            csum = small.tile([B, 1], f32, tag="csum")
            nc.vector.tensor_reduce(csum[:], c_log[:], axis=Ax.X, op=Alu.add)
            # normalize
            for ck in range(S // TCK):
                sl = slice(ck * TCK, (ck + 1) * TCK)
                nc.vector.tensor_mul(
                    preT3[:, sl, :],
                    preT3[:, sl, :],
                    rs_mask[:, sl].unsqueeze(2).broadcast_to([B, TCK, J]),
                )

            # ---------------- transpose to scan space ----------------
            idn_i = small.tile([B, B], i32, tag="idn_i")
            nc.gpsimd.iota(idn_i[:], pattern=[[1, B]], base=0,
                           channel_multiplier=-1)
            idn = small.tile([B, B], f32, tag="idn")
            nc.vector.tensor_scalar(idn[:], idn_i[:], 0.0, None,
                                    op0=Alu.is_equal)
            idn128_i = small.tile([P, P], i32, tag="idn128_i")
            nc.gpsimd.iota(idn128_i[:], pattern=[[1, P]], base=0,
                           channel_multiplier=-1)
            idn128 = small.tile([P, P], f32, tag="idn128")
            nc.vector.tensor_scalar(idn128[:], idn128_i[:], 0.0, None,
                                    op0=Alu.is_equal)

            # escan[128, chunk, b]: chunk ck holds t=4ck..4ck+3 at row
            # offsets 0/32/64/96
            escan = big.tile([P, NCHUNK * B], f32, tag="escan")
            escan3 = escan[:].rearrange("p (n b) -> p n b", b=B)
            for ck in range(NCHUNK):
                t0 = ck * CHUNK_T
                src = preT3[:, t0 : t0 + CHUNK_T, :].rearrange(
                    "b s j -> b (s j)"
                )
                pt = ptrans.tile([P, B], f32, tag="pt")
                nc.tensor.matmul(pt[:], src, idn[:], is_transpose=True,
                                 start=True, stop=True)
                nc.scalar.copy(escan3[:, ck, :], pt[:])

            # omega row in (t, b) free layout, host-precomputed
            oslab = big.tile([1, S * B], f32, tag="oslab")
            nc.sync.dma_start(oslab[:], om_d.ap())

            # ---------------- weights W [JA, JA] (lhsT layout) -------------
            W = small.tile([JA, JA], f32, tag="W")
            nc.vector.memset(W[:], 0.0)
            nc.sync.dma_start(W[:J, :J], T_d.ap())
            nc.sync.dma_start(W[:J, J : J + 1], en_d.ap())
            nc.scalar.activation(W[:J, :], W[:J, :], Exp)
            nc.vector.memset(W[J : J + 1, J : J + 1], 1.0)

            # expStart [J, 1]
            est = small.tile([J, 1], f32, tag="est")
            nc.sync.dma_start(est[:], st_d.ap())
            nc.scalar.activation(est[:], est[:], Exp)
            # endp [JA, 1]: exp(end) rows 0..31, omega 1
            enp = small.tile([JA, 1], f32, tag="enp")
            nc.sync.dma_start(enp[:J, :], en_d.ap())
            nc.scalar.activation(enp[:J, :], enp[:J, :], Exp)
            nc.vector.memset(enp[J : J + 1, :], 1.0)
            ones_ja = small.tile([JA, 1], f32, tag="ones_ja")
            nc.vector.memset(ones_ja[:], 1.0)
            ones_b = small.tile([B, 1], f32, tag="ones_b")
            nc.vector.memset(ones_b[:], 1.0)
            ones_p = small.tile([P, 1], f32, tag="ones_p")
            nc.vector.memset(ones_p[:], 1.0)

            # ---------------- the scan ----------------
            def e_ap(t):
                return escan3[J * (t % CHUNK_T) : J * (t % CHUNK_T) + J,
                              t // CHUNK_T, :]

            def o_ap(t):
                return oslab[0:1, t * B : (t + 1) * B]

            a_prev = apool.tile([JA, B], f32, tag="a")
            nc.vector.tensor_scalar(
                a_prev[:J, :], e_ap(0), est[:], None, op0=Alu.mult
            )
            nc.vector.memset(a_prev[J : J + 1, :], 0.0)
            for t in range(1, scan_steps):
                ps = pscan.tile([JA, B], f32, tag="ps")
                nc.tensor.matmul(ps[:], W[:], a_prev[:], start=True, stop=True)
                a_t = apool.tile([JA, B], f32, tag="a")
                nc.vector.tensor_mul(a_t[:J, :], ps[:J, :], e_ap(t))
                nc.vector.tensor_mul(a_t[J : J + 1, :], ps[J : J + 1, :],
                                     o_ap(t))
                a_prev = a_t

            # ---------------- finalize log-partition ----------------
            af = apool.tile([JA, B], f32, tag="af")
            nc.vector.tensor_scalar(af[:], a_prev[:], enp[:], None,
                                    op0=Alu.mult)
            zrow = pfin.tile([1, B], f32, tag="fin_a")
            nc.tensor.matmul(zrow[:], ones_ja[:], af[:], start=True, stop=True)
            logz = small.tile([1, B], f32, tag="logz")
            nc.scalar.activation(logz[:], zrow[:], Ln)

            if with_score:
                # ============ log-score (batch-summed, no gathers) ============
                # [128, 64] relayout: partition p covers b = p//8,
                # t in [(p%8)*64, (p%8)*64+64)
                em128 = big.tile([P, FS * J], f32, tag="em128")
                nc.sync.dma_start(
                    em128[:], em_d.ap().rearrange("b s j -> (b s j)")
                    .rearrange("(p f) -> p f", p=P)
                )
                tgt128 = small.tile([P, FS], i32, tag="tgt128")
                nc.sync.dma_start(
                    tgt128[:], tgt_d.ap().rearrange("b s -> (b s)")
                    .rearrange("(p f) -> p f", p=P)
                )
                tgt128f = small.tile([P, FS], f32, tag="tgt128f")
                nc.vector.tensor_copy(tgt128f[:], tgt128[:])
                # shifted targets: tgt[b, t+1] at (p, f); last element garbage
                # but always masked (t=511 pair is never valid)
                tgtn128 = small.tile([P, FS], i32, tag="tgtn128")
                tgt_flat = tgt_d.ap().rearrange("b s -> (b s)")
                tgtv = tgt_flat.rearrange("(p f) -> p f", p=P)
                nc.vector.memset(tgtn128[:, FS - 1 : FS], 0)
                nc.sync.dma_start(tgtn128[:, : FS - 1], tgtv[:, 1:])
                nc.sync.dma_start(tgtn128[: P - 1, FS - 1 : FS], tgtv[1:, 0:1])
                tgtn128f = small.tile([P, FS], f32, tag="tgtn128f")
                nc.vector.tensor_copy(tgtn128f[:], tgtn128[:])
                # masks reshaped via SBUF->SBUF DMA
                # t-index and length in the [128, 64] layout, computed on-chip
                i64 = small.tile([P, FS], i32, tag="i64")
                nc.gpsimd.iota(i64[:], pattern=[[1, FS]], base=0,
                               channel_multiplier=FS)
                piota = small.tile([P, 1], i32, tag="piota")
                nc.gpsimd.iota(piota[:], pattern=[[0, 1]], base=0,
                               channel_multiplier=1)
                bq = small.tile([P, 1], i32, tag="bq")
                nc.vector.tensor_scalar(bq[:], piota[:], 3, None,
                                        op0=Alu.arith_shift_right)
                boff = small.tile([P, 1], i32, tag="boff")
                nc.vector.tensor_scalar(boff[:], bq[:], 9, None,
                                        op0=Alu.logical_shift_left)
                bofff = small.tile([P, 1], f32, tag="bofff")
                nc.vector.tensor_copy(bofff[:], boff[:])
                tv128 = small.tile([P, FS], f32, tag="tv128")
                nc.vector.tensor_copy(tv128[:], i64[:])
                nc.vector.tensor_scalar(tv128[:], tv128[:], bofff[:], None,
                                        op0=Alu.subtract)
                # len128[p] = len[p//8] via a one-hot matmul broadcast
                bqf = small.tile([P, 1], f32, tag="bqf")
                nc.vector.tensor_copy(bqf[:], bq[:])
                iota16 = small.tile([P, B], i32, tag="iota16")
                nc.gpsimd.iota(iota16[:], pattern=[[1, B]], base=0,
                               channel_multiplier=0)
                iota16f = small.tile([P, B], f32, tag="iota16f")
                nc.vector.tensor_copy(iota16f[:], iota16[:])
                b8t = small.tile([P, B], f32, tag="b8t")
                nc.vector.tensor_scalar(b8t[:], iota16f[:], bqf[:], None,
                                        op0=Alu.is_equal)
                pb8 = ptrans.tile([B, P], f32, tag="pt")
                nc.tensor.matmul(pb8[:], b8t[:], idn128[:], is_transpose=True,
                                 start=True, stop=True)
                b8 = small.tile([B, P], f32, tag="b8")
                nc.scalar.copy(b8[:], pb8[:])
                pl128 = pfin.tile([P, 1], f32, tag="fin_a")
                nc.tensor.matmul(pl128[:], b8[:], len_f[:], start=True, stop=True)
                len128 = small.tile([P, 1], f32, tag="len128")
                nc.scalar.copy(len128[:], pl128[:])
                len128m1 = small.tile([P, 1], f32, tag="len128m1")
                nc.vector.tensor_scalar(len128m1[:], len128[:], -1.0, None,
                                        op0=Alu.add)
                mask128 = small.tile([P, FS], f32, tag="mask128")
                nc.vector.tensor_scalar(mask128[:], tv128[:], len128[:], None,
                                        op0=Alu.is_lt)
                maskn128 = small.tile([P, FS], f32, tag="maskn128")
                nc.vector.tensor_scalar(maskn128[:], tv128[:], len128m1[:], None,
                                        op0=Alu.is_lt)
                last128 = small.tile([P, FS], f32, tag="last128")
                nc.vector.tensor_scalar(last128[:], tv128[:], len128m1[:], None,
                                        op0=Alu.is_equal)
                # masked target codes: tgt where valid else -1
                tgtmP = small.tile([P, FS], f32, tag="tgtmP")
                nc.vector.tensor_scalar(tgtmP[:], tgt128f[:], 1.0, None,
                                        op0=Alu.add)
                nc.vector.tensor_mul(tgtmP[:], tgtmP[:], mask128[:])
                nc.vector.tensor_scalar(tgtmP[:], tgtmP[:], -1.0, None,
                                        op0=Alu.add)
                tgtmN = small.tile([P, FS], f32, tag="tgtmN")
                nc.vector.tensor_scalar(tgtmN[:], tgtn128f[:], 1.0, None,
                                        op0=Alu.add)
                nc.vector.tensor_mul(tgtmN[:], tgtmN[:], maskn128[:])
                nc.vector.tensor_scalar(tgtmN[:], tgtmN[:], -1.0, None,
                                        op0=Alu.add)
                # one-hot slabs [P, FS, J] via small iota broadcast along f
                iota_ji = small.tile([P, J], i32, tag="iota_ji")
                nc.gpsimd.iota(iota_ji[:], pattern=[[1, J]], base=0,
                               channel_multiplier=0)
                iota_jf = small.tile([P, J], f32, tag="iota_jf")
                nc.vector.tensor_copy(iota_jf[:], iota_ji[:])
                iota_b = (iota_jf[:].unsqueeze(1)
                          .broadcast_to([P, FS, J]))
                ohp = big.tile([P, FS * J], f32, tag="ohp")
                nc.vector.tensor_tensor(
                    ohp[:].rearrange("p (f j) -> p f j", j=J),
                    iota_b,
                    tgtmP[:].unsqueeze(2).broadcast_to([P, FS, J]),
                    op=Alu.is_equal,
                )
                ohn = big.tile([P, FS * J], f32, tag="ohn")
                nc.vector.tensor_tensor(
                    ohn[:].rearrange("p (f j) -> p f j", j=J),
                    iota_b,
                    tgtmN[:].unsqueeze(2).broadcast_to([P, FS, J]),
                    op=Alu.is_equal,
                )
                # transition count matrix C[i,j] over all valid pairs
                ohp3 = ohp[:].rearrange("p (f j) -> p f j", j=J)
                ohn3 = ohn[:].rearrange("p (f j) -> p f j", j=J)
                cpsum = pfin.tile([J, J], f32, tag="fin_a")
                with tc.tile_critical():
                    for f in range(FS):
                        nc.tensor.matmul(cpsum[:], ohp3[:, f, :], ohn3[:, f, :],
                                         start=(f == 0), stop=(f == FS - 1))
                Traw = small.tile([J, J], f32, tag="Traw")
                nc.sync.dma_start(Traw[:], T_d.ap())
                tsc = small.tile([J, 1], f32, tag="tsc")
                tscratch = small.tile([J, J], f32, tag="tscratch")
                nc.vector.tensor_mul(tscratch[:], cpsum[:], Traw[:])
                nc.vector.tensor_reduce(tsc[:], tscratch[:], axis=Ax.X, op=Alu.add)
                # end term: weights = sum over (p,f) of ohp * last128 -> [J]
                # (reuse ohn slab slot is not needed; overwrite ohn in place)
                wsel = ohn  # reuse the ohn slab after the C matmuls consumed it
                nc.vector.tensor_tensor(
                    wsel[:].rearrange("p (f j) -> p f j", j=J),
                    ohp3,
                    last128[:].unsqueeze(2).broadcast_to([P, FS, J]),
                    op=Alu.mult,
                )
                wselred = small.tile([P, J], f32, tag="wselred")
                nc.vector.tensor_reduce(
                    wselred[:],
                    wsel[:].rearrange("p (f j) -> p j f", j=J),
                    axis=Ax.X, op=Alu.add,
                )
                endcnt = pfin.tile([J, 1], f32, tag="fin_a")
                nc.tensor.matmul(endcnt[:], wselred[:], ones_p[:], start=True,
                                 stop=True)
                en_raw = small.tile([J, 1], f32, tag="en_raw")
                nc.sync.dma_start(en_raw[:], en_d.ap())
                endsc = small.tile([J, 1], f32, tag="endsc")
                nc.vector.tensor_mul(endsc[:], endcnt[:], en_raw[:])
                # emission term: sum(ohp * em128) -- in-place over ohp
                nc.vector.tensor_mul(ohp[:], ohp[:], em128[:])
                emred = small.tile([P, 1], f32, tag="emred")
                nc.vector.tensor_reduce(emred[:], ohp[:], axis=Ax.X, op=Alu.add)
                emtot = pfin.tile([1, 1], f32, tag="fin_a")
                nc.tensor.matmul(emtot[:], emred[:], ones_p[:], start=True,
                                 stop=True)
                # start term: counts of tgt[b, 0]
                tgt0 = small.tile([B, 1], i32, tag="tgt0")
                nc.sync.dma_start(tgt0[:], tgt_d.ap()[:, 0:1])
                tgt0f = small.tile([B, 1], f32, tag="tgt0f")
                nc.vector.tensor_copy(tgt0f[:], tgt0[:])
                iota_jb = small.tile([B, J], i32, tag="iota_jb")
                nc.gpsimd.iota(iota_jb[:], pattern=[[1, J]], base=0,
                               channel_multiplier=0)
                iota_jbf = small.tile([B, J], f32, tag="iota_jbf")
                nc.vector.tensor_copy(iota_jbf[:], iota_jb[:])
                oh0 = small.tile([B, J], f32, tag="oh0")
                nc.vector.tensor_scalar(oh0[:], iota_jbf[:], tgt0f[:], None,
                                        op0=Alu.is_equal)
                cnt0 = pfin.tile([J, 1], f32, tag="fin_a")
                nc.tensor.matmul(cnt0[:], oh0[:], ones_b[:], start=True, stop=True)
                st_raw = small.tile([J, 1], f32, tag="st_raw")
                nc.sync.dma_start(st_raw[:], st_d.ap())
                stsc = small.tile([J, 1], f32, tag="stsc")
                nc.vector.tensor_mul(stsc[:], cnt0[:], st_raw[:])

            # ---------------- combine ----------------
            # NLL = sum_b logz + sum_b csum - (emtot + sum(tsc+endsc+stsc))
            s_all = pfin.tile([1, 1], f32, tag="fin_a")
            nc.tensor.matmul(s_all[:], ones_b[:], csum[:], start=True,
                             stop=True)
            s2 = small.tile([1, 1], f32, tag="s2")
            nc.vector.tensor_reduce(s2[:], logz[:], axis=Ax.X, op=Alu.add)
            res = small.tile([1, 1], f32, tag="res")
            nc.vector.tensor_add(res[:], s_all[:], s2[:])
            if with_score:
                sneg = small.tile([J, 1], f32, tag="sneg")
                nc.vector.tensor_add(sneg[:], tsc[:], endsc[:])
                nc.vector.tensor_add(sneg[:], sneg[:], stsc[:])
                nc.vector.tensor_scalar(sneg[:], sneg[:], -1.0, None,
                                        op0=Alu.mult)
                ones_j = small.tile([J, 1], f32, tag="ones_j")
                nc.vector.memset(ones_j[:], 1.0)
                s3p = pfin.tile([1, 1], f32, tag="fin_a")
                nc.tensor.matmul(s3p[:], ones_j[:], sneg[:], start=True,
                                 stop=True)
                nc.vector.tensor_add(res[:], res[:], s3p[:])
                nc.vector.tensor_sub(res[:], res[:], emtot[:])
            nc.sync.dma_start(out_d.ap(), res[:])

    nc.compile()
    return nc


_NC_CACHE = None


def kernel(emission, length, target, transition, start_transition,
           end_transition):
    global _NC_CACHE
    from concourse.bass_utils import run_bass_kernel_spmd

    emission = np.ascontiguousarray(np.asarray(emission, np.float32))
    length = np.asarray(length).astype(np.int32).reshape(-1, 1)
    target = np.asarray(target).astype(np.int32)
    transition = np.ascontiguousarray(np.asarray(transition, np.float32))
    start = np.asarray(start_transition, np.float32).reshape(J, 1)
    end = np.asarray(end_transition, np.float32).reshape(J, 1)

    if _NC_CACHE is None:
        _NC_CACHE = build_bass()
    nc = _NC_CACHE

    tgrid = np.arange(S)[:, None]
    in_maps = []
    for c in range(NCORES):
        sl = slice(c * B, (c + 1) * B)
        om = (tgrid >= length[sl].reshape(1, B)).astype(np.float32)
        in_maps.append({
            "omega": np.ascontiguousarray(om.reshape(1, S * B)),
            "emission": np.ascontiguousarray(emission[sl]),
            "length": np.ascontiguousarray(length[sl]),
            "target": np.ascontiguousarray(target[sl]),
            "transition": transition,
            "start_transition": start,
            "end_transition": end,
        })

    r = run_bass_kernel_spmd(nc, in_maps, list(range(NCORES)))
    total = np.float64(0.0)
    for c in range(NCORES):
        total += np.float64(r.results[c]["out"][0, 0])
    return np.asarray(total, np.float32)


if __name__ == "__main__":
    rng = np.random.default_rng(0)
    inputs = {
        "emission": rng.standard_normal((128, S, J)).astype(np.float32),
        "length": rng.integers(2, S + 1, size=(128,)),
        "target": rng.integers(0, J, size=(128, S)),
        "transition": (rng.standard_normal((J, J)) * 0.1).astype(np.float32),
        "start_transition": (rng.standard_normal(J) * 0.1).astype(np.float32),
        "end_transition": (rng.standard_normal(J) * 0.1).astype(np.float32),
    }
    print(kernel(**inputs))



# revision 8
# speedup vs baseline: 2.5381x; 2.5381x over previous
"""CRF negative-log-likelihood kernel for Trainium2 (Bass/Tile), 8-core SPMD.

Problem: emission [128, 512, 32] f32, length [128], target [128, 512],
transition [32, 32], start/end_transition [32] -> scalar f32
  sum_b (log_partition_b - log_score_b)

Strategy (data-parallel over batch, 16 sequences per core):
  * log_partition via the forward algorithm in EXP space: each step is one
    matmul plus one elementwise multiply:  A_t = (W^T A_{t-1}) .* E_t with
    per-(t,b) pre-normalized E_t = exp(em_t)/sum_j exp(em_t); the dropped
    log-scales are accumulated separately and added back at the end.
  * Variable lengths use an absorbing extra tag "omega" (index 32):
    W[i, omega] = exp(end_i), W[omega, omega] = 1; the omega multiplier is
    1 on padded steps / 0 on real steps and lives as a 33rd row of the E
    tile so each scan step is a single [33, 16] elementwise multiply.
  * Meet-in-the-middle: a forward chain over t = 1..256 and an independent
    backward chain B_{t-1} = W (E_t .* B_t) over t = 511..257 run
    concurrently (both latency-bound, so they interleave on the engines);
    Z_b = sum_j A_256[j] B_256[j].  This halves the serial chain length.
  * All prep runs in a [128, 64*f] layout (partition p = b*8 + s8 covers
    t in [s8*64, s8*64+64)), so exp / rowsum / normalize sweeps use all
    128 partitions.  E tiles for the scan come from 22 big PE transposes
    of a [128, 64*33] omega-augmented slab (3 timesteps x 8 s8-blocks per
    transpose).
  * log_score needs only its SUM over the batch, so it is computed with
    one-hot / count-matrix contractions (no gathers) in the same [128, 64]
    layout; it shares the raw emission slab and masks with the prep.
  * Each core writes one partial sum; the host adds the 8 partials.
"""

import numpy as np

B = 16           # batch per core
S = 512          # sequence length
J = 32           # tags
JA = J + 1       # augmented with omega
NCORES = 8
P = 128          # full partition count for the [128, .] relayout
FS = B * S // P  # 64 free elems per partition in that relayout
NS8 = P // B     # 8 s8 blocks per sequence
KPAD = 64        # per-timestep column pad (tag rows must start 32-aligned)
TPC = 2          # timesteps per transpose chunk (offsets 0 and 64)
NCK = 32         # 64 / 2 transpose chunks
HALF = S // 2    # forward chain does t=1..HALF, backward t=S-1..HALF+1


def build_bass(with_score=True, mult_engine="vector"):
    import concourse.bacc as bacc
    import concourse.tile as tile
    from concourse import mybir

    f32 = mybir.dt.float32
    i32 = mybir.dt.int32

    nc = bacc.Bacc(
        "TRN2", target_bir_lowering=False, debug=False, num_devices=NCORES
    )

    em_d = nc.dram_tensor("emission", [B, S, J], f32, kind="ExternalInput")
    len_d = nc.dram_tensor("length", [B, 1], i32, kind="ExternalInput")
    tgt_d = nc.dram_tensor("target", [B, S], i32, kind="ExternalInput")
    T_d = nc.dram_tensor("transition", [J, J], f32, kind="ExternalInput")
    st_d = nc.dram_tensor("start_transition", [J, 1], f32, kind="ExternalInput")
    en_d = nc.dram_tensor("end_transition", [J, 1], f32, kind="ExternalInput")
    out_d = nc.dram_tensor("out", [1, 1], f32, kind="ExternalOutput")

    Exp = mybir.ActivationFunctionType.Exp
    Ln = mybir.ActivationFunctionType.Ln
    Alu = mybir.AluOpType
    Ax = mybir.AxisListType

    with tile.TileContext(nc) as tc:
        with (
            tc.tile_pool(name="big", bufs=1) as big,        # persistent slabs
            tc.tile_pool(name="small", bufs=1) as small,    # persistent small
            tc.tile_pool(name="af", bufs=3) as afp,         # fwd scan state
            tc.tile_pool(name="ab", bufs=3) as abp,         # bwd scan state
            tc.tile_pool(name="mb", bufs=3) as mbp,         # bwd multiplied vec
            tc.tile_pool(name="psf", bufs=2, space="PSUM") as psf,
            tc.tile_pool(name="psb", bufs=2, space="PSUM") as psb,
            tc.tile_pool(name="ptrans", bufs=2, space="PSUM") as ptrans,
            tc.tile_pool(name="pfin", bufs=2, space="PSUM") as pfin,
        ):
            # ---------------- tiny loads + iotas ----------------
            len_i = small.tile([B, 1], i32, tag="len_i")
            nc.sync.dma_start(len_i[:], len_d.ap())
            len_f = small.tile([B, 1], f32, tag="len_f")
            nc.vector.tensor_copy(len_f[:], len_i[:])

            idn128_i = small.tile([P, P], i32, tag="idn128_i")
            nc.gpsimd.iota(idn128_i[:], pattern=[[1, P]], base=0,
                           channel_multiplier=-1)
            idn128 = small.tile([P, P], f32, tag="idn128")
            nc.vector.tensor_scalar(idn128[:], idn128_i[:], 0.0, None,
                                    op0=Alu.is_equal)
            idn33_i = small.tile([JA, JA], i32, tag="idn33_i")
            nc.gpsimd.iota(idn33_i[:], pattern=[[1, JA]], base=0,
                           channel_multiplier=-1)
            idn33 = small.tile([JA, JA], f32, tag="idn33")
            nc.vector.tensor_scalar(idn33[:], idn33_i[:], 0.0, None,
                                    op0=Alu.is_equal)

            # ---------------- [128, 64] masks ----------------
            # partition p covers b = p//8, t in [(p%8)*64, (p%8)*64 + 64)
            i64 = small.tile([P, FS], i32, tag="i64")
            nc.gpsimd.iota(i64[:], pattern=[[1, FS]], base=0,
                           channel_multiplier=FS)
            piota = small.tile([P, 1], i32, tag="piota")
            nc.gpsimd.iota(piota[:], pattern=[[0, 1]], base=0,
                           channel_multiplier=1)
            bq = small.tile([P, 1], i32, tag="bq")
            nc.vector.tensor_scalar(bq[:], piota[:], 3, None,
                                    op0=Alu.arith_shift_right)
            boff = small.tile([P, 1], i32, tag="boff")
            nc.vector.tensor_scalar(boff[:], bq[:], 9, None,
                                    op0=Alu.logical_shift_left)
            bofff = small.tile([P, 1], f32, tag="bofff")
            nc.vector.tensor_copy(bofff[:], boff[:])
            tv128 = small.tile([P, FS], f32, tag="tv128")
            nc.vector.tensor_copy(tv128[:], i64[:])
            nc.vector.tensor_scalar(tv128[:], tv128[:], bofff[:], None,
                                    op0=Alu.subtract)
            # len128[p] = len[p//8] via a one-hot matmul broadcast
            bqf = small.tile([P, 1], f32, tag="bqf")
            nc.vector.tensor_copy(bqf[:], bq[:])
            iota16 = small.tile([P, B], i32, tag="iota16")
            nc.gpsimd.iota(iota16[:], pattern=[[1, B]], base=0,
                           channel_multiplier=0)
            iota16f = small.tile([P, B], f32, tag="iota16f")
            nc.vector.tensor_copy(iota16f[:], iota16[:])
            b8t = small.tile([P, B], f32, tag="b8t")
            nc.vector.tensor_scalar(b8t[:], iota16f[:], bqf[:], None,
                                    op0=Alu.is_equal)
            # permutation matrix Pi[p, c] = 1 iff c = (p%8)*16 + p//8, so a
            # plain matmul src^T @ Pi lands transpose columns in (s8, b)
            # order and E tiles for one timestep are contiguous 16-col runs
            bmul = small.tile([P, 1], i32, tag="bmul")
            nc.vector.tensor_scalar(bmul[:], bq[:], 3, None,
                                    op0=Alu.logical_shift_left)
            s8i = small.tile([P, 1], i32, tag="s8i")
            nc.vector.tensor_sub(s8i[:], piota[:], bmul[:])
            permv = small.tile([P, 1], i32, tag="permv")
            nc.vector.tensor_scalar(permv[:], s8i[:], 4, None,
                                    op0=Alu.logical_shift_left)
            nc.vector.tensor_add(permv[:], permv[:], bq[:])
            permvf = small.tile([P, 1], f32, tag="permvf")
            nc.vector.tensor_copy(permvf[:], permv[:])
            iotap = small.tile([P, P], i32, tag="iotap")
            nc.gpsimd.iota(iotap[:], pattern=[[1, P]], base=0,
                           channel_multiplier=0)
            iotapf = small.tile([P, P], f32, tag="iotapf")
            nc.vector.tensor_copy(iotapf[:], iotap[:])
            perm = small.tile([P, P], f32, tag="perm")
            nc.vector.tensor_scalar(perm[:], iotapf[:], permvf[:], None,
                                    op0=Alu.is_equal)
            pb8 = ptrans.tile([B, P], f32, tag="pt")
            nc.tensor.matmul(pb8[:], b8t[:], idn128[:], is_transpose=True,
                             start=True, stop=True)
            b8 = small.tile([B, P], f32, tag="b8")
            nc.scalar.copy(b8[:], pb8[:])
            pl128 = pfin.tile([P, 1], f32, tag="fin_a")
            nc.tensor.matmul(pl128[:], b8[:], len_f[:], start=True, stop=True)
            len128 = small.tile([P, 1], f32, tag="len128")
            nc.scalar.copy(len128[:], pl128[:])
            mask128 = small.tile([P, FS], f32, tag="mask128")
            nc.vector.tensor_scalar(mask128[:], tv128[:], len128[:], None,
                                    op0=Alu.is_lt)

            # ---------------- weights ----------------
            W = small.tile([JA, JA], f32, tag="W")
            nc.vector.memset(W[:], 0.0)
            nc.sync.dma_start(W[:J, :J], T_d.ap())
            nc.sync.dma_start(W[:J, J : J + 1], en_d.ap())
            nc.scalar.activation(W[:J, :], W[:J, :], Exp)
            nc.vector.memset(W[J : J + 1, J : J + 1], 1.0)
            # WT = transpose(W) so that matmul(., WT, x) = W @ x
            ptw = ptrans.tile([JA, JA], f32, tag="pt")
            nc.tensor.matmul(ptw[:], W[:], idn33[:], is_transpose=True,
                             start=True, stop=True)
            WT = small.tile([JA, JA], f32, tag="WT")
            nc.scalar.copy(WT[:], ptw[:])

            est = small.tile([J, 1], f32, tag="est")
            nc.sync.dma_start(est[:], st_d.ap())
            nc.scalar.activation(est[:], est[:], Exp)
            enp = small.tile([JA, 1], f32, tag="enp")
            nc.sync.dma_start(enp[:J, :], en_d.ap())
            nc.scalar.activation(enp[:J, :], enp[:J, :], Exp)
            nc.vector.memset(enp[J : J + 1, :], 1.0)
            ones_ja = small.tile([JA, 1], f32, tag="ones_ja")
            nc.vector.memset(ones_ja[:], 1.0)
            ones_p = small.tile([P, 1], f32, tag="ones_p")
            nc.vector.memset(ones_p[:], 1.0)
            ones_b = small.tile([B, 1], f32, tag="ones_b")
            nc.vector.memset(ones_b[:], 1.0)

            # ---------------- emission prep, [128, .] layout ----------------
            # emraw[p, f*J + j] = em[b, t, j], p = b*8 + s8, t = s8*64 + f
            emraw = big.tile([P, FS * J], f32, tag="emraw")
            expP = big.tile([P, FS * J], f32, tag="expP")
            ssum = small.tile([P, FS], f32, tag="ssum")
            s_eff = small.tile([P, FS], f32, tag="s_eff")
            rs128 = small.tile([P, FS], f32, tag="rs128")
            c_log = small.tile([P, FS], f32, tag="c_log")
            # emA: exp-normalized emissions augmented with the omega
            # multiplier as tag 32, padded to 64 cols per timestep so the
            # transposed E tiles land at partition offsets 0/64 (32-aligned)
            emA = big.tile([P, FS * KPAD], f32, tag="emA")
            emA3 = emA[:].rearrange("p (f k) -> p f k", k=KPAD)
            nc.gpsimd.memset(emA[:], 0.0)
            emraw3 = emraw[:].rearrange("p (f j) -> p f j", j=J)
            expP3 = expP[:].rearrange("p (f j) -> p f j", j=J)
            em_flat = em_d.ap().rearrange("b s j -> (b s j)").rearrange(
                "(p f) -> p f", p=P
            )

            # process in f-quarters so the scan chains can start early;
            # order 0,3,1,2 so both chain heads (t=1 needs f=0..2, t=511
            # needs f=63) are fed first.
            FQ = FS // 4
            for q in (0, 3, 1, 2):
                fsl = slice(q * FQ, (q + 1) * FQ)
                csl = slice(q * FQ * J, (q + 1) * FQ * J)
                nc.sync.dma_start(emraw[:, csl], em_flat[:, csl])
                nc.scalar.activation(expP[:, csl], emraw[:, csl], Exp)
                nc.vector.tensor_reduce(
                    ssum[:, fsl], expP3[:, fsl, :], axis=Ax.X, op=Alu.add
                )
                # s_eff = (ssum - 1) * mask + 1   (=1 on padded steps)
                nc.vector.tensor_scalar(
                    s_eff[:, fsl], ssum[:, fsl], -1.0, None, op0=Alu.add
                )
                nc.vector.tensor_mul(s_eff[:, fsl], s_eff[:, fsl],
                                     mask128[:, fsl])
                nc.vector.tensor_scalar(
                    s_eff[:, fsl], s_eff[:, fsl], 1.0, None, op0=Alu.add
                )
                # rs = mask / s_eff ; c_log = ln(s_eff)
                nc.vector.reciprocal(rs128[:, fsl], s_eff[:, fsl])
                nc.vector.tensor_mul(rs128[:, fsl], rs128[:, fsl],
                                     mask128[:, fsl])
                nc.scalar.activation(c_log[:, fsl], s_eff[:, fsl], Ln)
                # normalized exp into the augmented slab
                nc.vector.tensor_tensor(
                    emA3[:, fsl, :J],
                    expP3[:, fsl, :],
                    rs128[:, fsl].unsqueeze(2).broadcast_to([P, FQ, J]),
                    op=Alu.mult,
                )
                # omega multiplier column: 1 - mask
                nc.vector.tensor_scalar(
                    emA3[:, fsl, J], mask128[:, fsl], -1.0, 1.0,
                    op0=Alu.mult, op1=Alu.add,
                )

            # csum total = sum over (p, f) of c_log
            csred = small.tile([P, 1], f32, tag="csred")
            nc.vector.tensor_reduce(csred[:], c_log[:], axis=Ax.X, op=Alu.add)

            # ---------------- transpose to scan layout ----------------
            # escan[sl*64 + j, ck, s8*16 + b] = emA[b*8+s8, (2*ck+sl)*64+j]
            # via plain matmul emA_slice^T @ perm (transpose + column perm)
            escan = big.tile([P, NCK * P], f32, tag="escan")
            escan3 = escan[:].rearrange("r (c p) -> r c p", p=P)
            # ping-pong order so both chain heads are fed immediately
            cks = []
            lo, hi = 0, NCK - 1
            while lo <= hi:
                cks.append(hi)
                if lo != hi:
                    cks.append(lo)
                hi -= 1
                lo += 1
            for ck in cks:
                pt = ptrans.tile([P, P], f32, tag="pt")
                nc.tensor.matmul(
                    pt[:], emA[:, ck * KPAD * TPC : (ck + 1) * KPAD * TPC],
                    perm[:], start=True, stop=True
                )
                nc.scalar.copy(escan3[:, ck, :], pt[:])

            def e_ap(t):
                s8, f = t // FS, t % FS
                ck, sl = f // TPC, f % TPC
                return escan3[sl * KPAD : sl * KPAD + JA, ck,
                              s8 * B : (s8 + 1) * B]

            mul_f = (nc.gpsimd.tensor_mul if mult_engine == "gpsimd"
                     else nc.vector.tensor_mul)

            # ---------------- the two scan chains ----------------
            # forward: A_t = (W^T A_{t-1}) .* E_t,  t = 1..HALF
            a_prev = afp.tile([JA, B], f32, tag="a")
            nc.vector.tensor_scalar(
                a_prev[:J, :], e_ap(0)[:J, :], est[:], None, op0=Alu.mult
            )
            nc.vector.memset(a_prev[J : J + 1, :], 0.0)
            # backward: B_{t-1} = W (E_t .* B_t),  t = S-1 .. HALF+1
            # B_init lives in PSUM (diag(enp) @ ones) so every backward
            # multiply is PSUM x SBUF — SB/SB ops would need equal base
            # partitions, which the 0/64-alternating E tiles violate.
            enpdiag = small.tile([JA, JA], f32, tag="enpdiag")
            nc.vector.tensor_scalar(enpdiag[:], idn33[:], enp[:], None,
                                    op0=Alu.mult)
            ones_jb = small.tile([JA, B], f32, tag="ones_jb")
            nc.vector.memset(ones_jb[:], 1.0)
            b_prev = psb.tile([JA, B], f32, tag="pb")
            nc.tensor.matmul(b_prev[:], enpdiag[:], ones_jb[:], start=True,
                             stop=True)

            for r in range(S - 1 - HALF):  # 255 paired rounds
                tf, tb = r + 1, S - 1 - r
                # forward step tf
                ps = psf.tile([JA, B], f32, tag="ps")
                nc.tensor.matmul(ps[:], W[:], a_prev[:], start=True, stop=True)
                a_t = afp.tile([JA, B], f32, tag="a")
                mul_f(a_t[:], ps[:], e_ap(tf))
                a_prev = a_t
                # backward step tb (b_prev stays in PSUM between rounds;
                # the next round's multiply drains it)
                m = mbp.tile([JA, B], f32, tag="m")
                mul_f(m[:], b_prev[:], e_ap(tb))
                pb = psb.tile([JA, B], f32, tag="pb")
                nc.tensor.matmul(pb[:], WT[:], m[:], start=True, stop=True)
                b_prev = pb
            # one extra forward step to reach t = HALF
            ps = psf.tile([JA, B], f32, tag="ps")
            nc.tensor.matmul(ps[:], W[:], a_prev[:], start=True, stop=True)
            a_t = afp.tile([JA, B], f32, tag="a")
            mul_f(a_t[:], ps[:], e_ap(HALF))
            a_prev = a_t

            # ---------------- combine: Z_b = sum_j A[j] B[j] ----------------
            v = small.tile([JA, B], f32, tag="v")
            nc.vector.tensor_mul(v[:], a_prev[:], b_prev[:])
            zrow = pfin.tile([1, B], f32, tag="fin_a")
            nc.tensor.matmul(zrow[:], ones_ja[:], v[:], start=True, stop=True)
            logz = small.tile([1, B], f32, tag="logz")
            nc.scalar.activation(logz[:], zrow[:], Ln)

            if with_score:
                # ============ log-score (batch-summed, no gathers) ==========
                len128m1 = small.tile([P, 1], f32, tag="len128m1")
                nc.vector.tensor_scalar(len128m1[:], len128[:], -1.0, None,
                                        op0=Alu.add)
                tgt128 = small.tile([P, FS], i32, tag="tgt128")
                tgt_flat = tgt_d.ap().rearrange("b s -> (b s)")
                tgtv = tgt_flat.rearrange("(p f) -> p f", p=P)
                nc.sync.dma_start(tgt128[:], tgtv)
                tgt128f = small.tile([P, FS], f32, tag="tgt128f")
                nc.vector.tensor_copy(tgt128f[:], tgt128[:])
                # shifted targets: tgt[b, t+1] at (p, f); garbage at the
                # seam is always masked (the t=511 pair is never valid)
                tgtn128 = small.tile([P, FS], i32, tag="tgtn128")
                nc.vector.memset(tgtn128[:, FS - 1 : FS], 0)
                nc.sync.dma_start(tgtn128[:, : FS - 1], tgtv[:, 1:])
                nc.sync.dma_start(tgtn128[: P - 1, FS - 1 : FS], tgtv[1:, 0:1])
                tgtn128f = small.tile([P, FS], f32, tag="tgtn128f")
                nc.vector.tensor_copy(tgtn128f[:], tgtn128[:])
                maskn128 = small.tile([P, FS], f32, tag="maskn128")
                nc.vector.tensor_scalar(maskn128[:], tv128[:], len128m1[:],
                                        None, op0=Alu.is_lt)
                last128 = small.tile([P, FS], f32, tag="last128")
                nc.vector.tensor_scalar(last128[:], tv128[:], len128m1[:],
                                        None, op0=Alu.is_equal)
                # masked target codes: tgt where valid else -1
                tgtmP = small.tile([P, FS], f32, tag="tgtmP")
                nc.vector.tensor_scalar(tgtmP[:], tgt128f[:], 1.0, None,
                                        op0=Alu.add)
                nc.vector.tensor_mul(tgtmP[:], tgtmP[:], mask128[:])
                nc.vector.tensor_scalar(tgtmP[:], tgtmP[:], -1.0, None,
                                        op0=Alu.add)
                tgtmN = small.tile([P, FS], f32, tag="tgtmN")
                nc.vector.tensor_scalar(tgtmN[:], tgtn128f[:], 1.0, None,
                                        op0=Alu.add)
                nc.vector.tensor_mul(tgtmN[:], tgtmN[:], maskn128[:])
                nc.vector.tensor_scalar(tgtmN[:], tgtmN[:], -1.0, None,
                                        op0=Alu.add)
                # one-hot slabs [P, FS, J]
                iota_ji = small.tile([P, J], i32, tag="iota_ji")
                nc.gpsimd.iota(iota_ji[:], pattern=[[1, J]], base=0,
                               channel_multiplier=0)
                iota_jf = small.tile([P, J], f32, tag="iota_jf")
                nc.vector.tensor_copy(iota_jf[:], iota_ji[:])
                iota_b = iota_jf[:].unsqueeze(1).broadcast_to([P, FS, J])
                ohp = big.tile([P, FS * J], f32, tag="ohp")
                ohp3 = ohp[:].rearrange("p (f j) -> p f j", j=J)
                nc.vector.tensor_tensor(
                    ohp3, iota_b,
                    tgtmP[:].unsqueeze(2).broadcast_to([P, FS, J]),
                    op=Alu.is_equal,
                )
                ohn = big.tile([P, FS * J], f32, tag="ohn")
                ohn3 = ohn[:].rearrange("p (f j) -> p f j", j=J)
                nc.vector.tensor_tensor(
                    ohn3, iota_b,
                    tgtmN[:].unsqueeze(2).broadcast_to([P, FS, J]),
                    op=Alu.is_equal,
                )
                # transition count matrix C[i,j] over all valid pairs
                cpsum = pfin.tile([J, J], f32, tag="fin_a")
                with tc.tile_critical():
                    for f in range(FS):
                        nc.tensor.matmul(cpsum[:], ohp3[:, f, :],
                                         ohn3[:, f, :],
                                         start=(f == 0), stop=(f == FS - 1))
                Traw = small.tile([J, J], f32, tag="Traw")
                nc.sync.dma_start(Traw[:], T_d.ap())
                tsc = small.tile([J, 1], f32, tag="tsc")
                tscratch = small.tile([J, J], f32, tag="tscratch")
                nc.vector.tensor_mul(tscratch[:], cpsum[:], Traw[:])
                nc.vector.tensor_reduce(tsc[:], tscratch[:], axis=Ax.X,
                                        op=Alu.add)
                # end term: weights = sum over (p,f) of ohp * last128 -> [J]
                wsel = ohn  # reuse after the C matmuls consumed it
                nc.vector.tensor_tensor(
                    wsel[:].rearrange("p (f j) -> p f j", j=J),
                    ohp3,
                    last128[:].unsqueeze(2).broadcast_to([P, FS, J]),
                    op=Alu.mult,
                )
                wselred = small.tile([P, J], f32, tag="wselred")
                nc.vector.tensor_reduce(
                    wselred[:],
                    wsel[:].rearrange("p (f j) -> p j f", j=J),
                    axis=Ax.X, op=Alu.add,
                )
                endcnt = pfin.tile([J, 1], f32, tag="fin_a")
                nc.tensor.matmul(endcnt[:], wselred[:], ones_p[:], start=True,
                                 stop=True)
                en_raw = small.tile([J, 1], f32, tag="en_raw")
                nc.sync.dma_start(en_raw[:], en_d.ap())
                endsc = small.tile([J, 1], f32, tag="endsc")
                nc.vector.tensor_mul(endsc[:], endcnt[:], en_raw[:])
                # emission term: sum(ohp * emraw) -- in-place over ohp
                nc.vector.tensor_mul(ohp[:], ohp[:], emraw[:])
                emred = small.tile([P, 1], f32, tag="emred")
                nc.vector.tensor_reduce(emred[:], ohp[:], axis=Ax.X,
                                        op=Alu.add)
                emtot = pfin.tile([1, 1], f32, tag="fin_a")
                nc.tensor.matmul(emtot[:], emred[:], ones_p[:], start=True,
                                 stop=True)
                # start term: counts of tgt[b, 0]
                tgt0 = small.tile([B, 1], i32, tag="tgt0")
                nc.sync.dma_start(tgt0[:], tgt_d.ap()[:, 0:1])
                tgt0f = small.tile([B, 1], f32, tag="tgt0f")
                nc.vector.tensor_copy(tgt0f[:], tgt0[:])
                iota_jb = small.tile([B, J], i32, tag="iota_jb")
                nc.gpsimd.iota(iota_jb[:], pattern=[[1, J]], base=0,
                               channel_multiplier=0)
                iota_jbf = small.tile([B, J], f32, tag="iota_jbf")
                nc.vector.tensor_copy(iota_jbf[:], iota_jb[:])
                oh0 = small.tile([B, J], f32, tag="oh0")
                nc.vector.tensor_scalar(oh0[:], iota_jbf[:], tgt0f[:], None,
                                        op0=Alu.is_equal)
                cnt0 = pfin.tile([J, 1], f32, tag="fin_a")
                nc.tensor.matmul(cnt0[:], oh0[:], ones_b[:], start=True,
                                 stop=True)
                st_raw = small.tile([J, 1], f32, tag="st_raw")
                nc.sync.dma_start(st_raw[:], st_d.ap())
                stsc = small.tile([J, 1], f32, tag="stsc")
                nc.vector.tensor_mul(stsc[:], cnt0[:], st_raw[:])

            # ---------------- combine ----------------
            # NLL = sum_b logz + sum(c_log) - (emtot + sum(tsc+endsc+stsc))
            s_all = pfin.tile([1, 1], f32, tag="fin_a")
            nc.tensor.matmul(s_all[:], csred[:], ones_p[:], start=True,
                             stop=True)
            s2 = small.tile([1, 1], f32, tag="s2")
            nc.vector.tensor_reduce(s2[:], logz[:], axis=Ax.X, op=Alu.add)
            res = small.tile([1, 1], f32, tag="res")
            nc.vector.tensor_add(res[:], s_all[:], s2[:])
            if with_score:
                sneg = small.tile([J, 1], f32, tag="sneg")
                nc.vector.tensor_add(sneg[:], tsc[:], endsc[:])
                nc.vector.tensor_add(sneg[:], sneg[:], stsc[:])
                nc.vector.tensor_scalar(sneg[:], sneg[:], -1.0, None,
                                        op0=Alu.mult)
                ones_j = small.tile([J, 1], f32, tag="ones_j")
                nc.vector.memset(ones_j[:], 1.0)
                s3p = pfin.tile([1, 1], f32, tag="fin_a")
                nc.tensor.matmul(s3p[:], ones_j[:], sneg[:], start=True,
                                 stop=True)
                nc.vector.tensor_add(res[:], res[:], s3p[:])
                nc.vector.tensor_sub(res[:], res[:], emtot[:])
            nc.sync.dma_start(out_d.ap(), res[:])

    nc.compile()
    return nc


_NC_CACHE = None


def kernel(emission, length, target, transition, start_transition,
           end_transition):
    global _NC_CACHE
    from concourse.bass_utils import run_bass_kernel_spmd

    emission = np.ascontiguousarray(np.asarray(emission, np.float32))
    length = np.asarray(length).astype(np.int32).reshape(-1, 1)
    target = np.asarray(target).astype(np.int32)
    transition = np.ascontiguousarray(np.asarray(transition, np.float32))
    start = np.asarray(start_transition, np.float32).reshape(J, 1)
    end = np.asarray(end_transition, np.float32).reshape(J, 1)

    if _NC_CACHE is None:
        _NC_CACHE = build_bass()
    nc = _NC_CACHE

    in_maps = []
    for c in range(NCORES):
        sl = slice(c * B, (c + 1) * B)
        in_maps.append({
            "emission": np.ascontiguousarray(emission[sl]),
            "length": np.ascontiguousarray(length[sl]),
            "target": np.ascontiguousarray(target[sl]),
            "transition": transition,
            "start_transition": start,
            "end_transition": end,
        })

    r = run_bass_kernel_spmd(nc, in_maps, list(range(NCORES)))
    total = np.float64(0.0)
    for c in range(NCORES):
        total += np.float64(r.results[c]["out"][0, 0])
    return np.asarray(total, np.float32)


if __name__ == "__main__":
    rng = np.random.default_rng(0)
    inputs = {
        "emission": rng.standard_normal((128, S, J)).astype(np.float32),
        "length": rng.integers(2, S + 1, size=(128,)),
        "target": rng.integers(0, J, size=(128, S)),
        "transition": (rng.standard_normal((J, J)) * 0.1).astype(np.float32),
        "start_transition": (rng.standard_normal(J) * 0.1).astype(np.float32),
        "end_transition": (rng.standard_normal(J) * 0.1).astype(np.float32),
    }
    print(kernel(**inputs))


# revision 16
# speedup vs baseline: 2.6961x; 1.0623x over previous
"""CRF negative-log-likelihood kernel for Trainium2 (Bass/Tile), 8-core SPMD.

Problem: emission [128, 512, 32] f32, length [128], target [128, 512],
transition [32, 32], start/end_transition [32] -> scalar f32
  sum_b (log_partition_b - log_score_b)

Strategy (data-parallel over batch, 16 sequences per core):
  * log_partition via the forward algorithm in EXP space: each step is one
    matmul plus one elementwise multiply:  A_t = (W^T A_{t-1}) .* E_t with
    per-(t,b) pre-normalized E_t = exp(em_t)/sum_j exp(em_t); the dropped
    log-scales are accumulated separately and added back at the end.
  * Variable lengths use an absorbing extra tag "omega" (index 32):
    W[i, omega] = exp(end_i), W[omega, omega] = 1; the omega multiplier is
    1 on padded steps / 0 on real steps and lives as a 33rd row of the E
    tile so each scan step is a single [33, 16] elementwise multiply.
  * Meet-in-the-middle: a forward chain over t = 1..256 and an independent
    backward chain B_{t-1} = W (E_t .* B_t) over t = 511..257 run
    concurrently (both latency-bound, so they interleave on the engines);
    Z_b = sum_j A_256[j] B_256[j].  This halves the serial chain length.
  * All prep runs in a [128, 64*k] layout (partition p = b*8 + s8 covers
    t in [s8*64, s8*64+64)), so exp / rowsum / normalize sweeps use all
    128 partitions.  E tiles come from 32 PE transposes (plain matmul with
    a column-permutation rhs) of a [128, 64*64] omega-augmented bf16 slab
    (2 timesteps x 8 s8-blocks per transpose, timesteps padded to 64 cols
    so tag rows land at the 32-aligned partition offsets 0/64).
  * log_score needs only its SUM over the batch: one-hot slabs (built on
    the otherwise-idle GPSIMD) are contracted against shifted one-hots /
    raw emissions / last-step masks by three PSUM-accumulating matmul
    groups WOVEN between the scan matmuls (PE is ~30% busy there), so the
    score costs almost no wall-clock; a tiny finalization runs after.
  * Each core writes one partial sum; the host adds the 8 partials.
"""

import numpy as np

B = 16           # batch per core
S = 512          # sequence length
J = 32           # tags
JA = J + 1       # augmented with omega
NCORES = 8
P = 128          # full partition count for the [128, .] relayout
FS = B * S // P  # 64 free elems per partition in that relayout
NS8 = P // B     # 8 s8 blocks per sequence
KPAD = 64        # per-timestep column pad (tag rows must start 32-aligned)
TPC = 2          # timesteps per transpose chunk (offsets 0 and 64)
NCK = 32         # 64 / 2 transpose chunks
HALF = S // 2    # forward chain does t=1..HALF, backward t=S-1..HALF+1
WSTART = 36      # first round that may carry a woven score matmul


def build_bass(with_score=True, rounds=S - 1 - HALF):
    import concourse.bacc as bacc
    import concourse.tile as tile
    from concourse import mybir

    f32 = mybir.dt.float32
    bf16 = mybir.dt.bfloat16
    i32 = mybir.dt.int32

    nc = bacc.Bacc(
        "TRN2", target_bir_lowering=False, debug=False, num_devices=NCORES
    )

    em_d = nc.dram_tensor("emission", [B, S, J], f32, kind="ExternalInput")
    len_d = nc.dram_tensor("length", [B, 1], i32, kind="ExternalInput")
    tgt_d = nc.dram_tensor("target", [B, S], i32, kind="ExternalInput")
    T_d = nc.dram_tensor("transition", [J, J], f32, kind="ExternalInput")
    st_d = nc.dram_tensor("start_transition", [J, 1], f32, kind="ExternalInput")
    en_d = nc.dram_tensor("end_transition", [J, 1], f32, kind="ExternalInput")
    out_d = nc.dram_tensor("out", [1, 1], f32, kind="ExternalOutput")

    Exp = mybir.ActivationFunctionType.Exp
    Ln = mybir.ActivationFunctionType.Ln
    Alu = mybir.AluOpType
    Ax = mybir.AxisListType

    with tile.TileContext(nc) as tc:
        with (
            tc.tile_pool(name="big", bufs=1) as big,        # persistent slabs
            tc.tile_pool(name="small", bufs=1) as small,    # persistent small
            tc.tile_pool(name="af", bufs=3) as afp,         # fwd scan state
            tc.tile_pool(name="mb", bufs=3) as mbp,         # bwd multiplied vec
            tc.tile_pool(name="psf", bufs=2, space="PSUM") as psf,
            tc.tile_pool(name="psb", bufs=2, space="PSUM") as psb,
            tc.tile_pool(name="ptrans", bufs=1, space="PSUM") as ptrans,
            tc.tile_pool(name="pfin", bufs=1, space="PSUM") as pfin,
        ):
            # ---------------- tiny loads + iotas ----------------
            len_i = small.tile([B, 1], i32, tag="len_i")
            nc.sync.dma_start(len_i[:], len_d.ap())
            len_f = small.tile([B, 1], f32, tag="len_f")
            nc.vector.tensor_copy(len_f[:], len_i[:])

            idn33_i = small.tile([JA, JA], i32, tag="idn33_i")
            nc.gpsimd.iota(idn33_i[:], pattern=[[1, JA]], base=0,
                           channel_multiplier=-1)
            idn33 = small.tile([JA, JA], f32, tag="idn33")
            nc.vector.tensor_scalar(idn33[:], idn33_i[:], 0.0, None,
                                    op0=Alu.is_equal)

            # ---------------- [128, 64] masks ----------------
            # partition p covers b = p//8, t in [(p%8)*64, (p%8)*64 + 64)
            i64 = small.tile([P, FS], i32, tag="i64")
            nc.gpsimd.iota(i64[:], pattern=[[1, FS]], base=0,
                           channel_multiplier=FS)
            piota = small.tile([P, 1], i32, tag="piota")
            nc.gpsimd.iota(piota[:], pattern=[[0, 1]], base=0,
                           channel_multiplier=1)
            bq = small.tile([P, 1], i32, tag="bq")
            nc.vector.tensor_scalar(bq[:], piota[:], 3, None,
                                    op0=Alu.arith_shift_right)
            boff = small.tile([P, 1], i32, tag="boff")
            nc.vector.tensor_scalar(boff[:], bq[:], 9, None,
                                    op0=Alu.logical_shift_left)
            bofff = small.tile([P, 1], f32, tag="bofff")
            nc.vector.tensor_copy(bofff[:], boff[:])
            tv128 = small.tile([P, FS], f32, tag="tv128")
            nc.vector.tensor_copy(tv128[:], i64[:])
            nc.vector.tensor_scalar(tv128[:], tv128[:], bofff[:], None,
                                    op0=Alu.subtract)
            # len128[p] = len[p//8] via a one-hot matmul broadcast:
            # b8T[b, p] = 1 iff p>>3 == b, then len128 = b8T^T... (lhsT)
            iotab_i = small.tile([B, P], i32, tag="iotab_i")
            nc.gpsimd.iota(iotab_i[:], pattern=[[1, P]], base=0,
                           channel_multiplier=0)
            pq_i = small.tile([B, P], i32, tag="pq_i")
            nc.vector.tensor_scalar(pq_i[:], iotab_i[:], 3, None,
                                    op0=Alu.arith_shift_right)
            pq_f = small.tile([B, P], f32, tag="pq_f")
            nc.vector.tensor_copy(pq_f[:], pq_i[:])
            biota = small.tile([B, 1], i32, tag="biota")
            nc.gpsimd.iota(biota[:], pattern=[[0, 1]], base=0,
                           channel_multiplier=1)
            biotaf = small.tile([B, 1], f32, tag="biotaf")
            nc.vector.tensor_copy(biotaf[:], biota[:])
            b8T = small.tile([B, P], f32, tag="b8T")
            nc.vector.tensor_scalar(b8T[:], pq_f[:], biotaf[:], None,
                                    op0=Alu.is_equal)
            pl128 = pfin.tile([P, 1], f32, tag="fin_a")
            nc.tensor.matmul(pl128[:], b8T[:], len_f[:], start=True, stop=True)
            len128 = small.tile([P, 1], f32, tag="len128")
            nc.scalar.copy(len128[:], pl128[:])
            mask128 = small.tile([P, FS], f32, tag="mask128")
            nc.vector.tensor_scalar(mask128[:], tv128[:], len128[:], None,
                                    op0=Alu.is_lt)

            # permutation matrix Pi[p, c] = 1 iff c = (p%8)*16 + p//8, so a
            # plain matmul src^T @ Pi lands transpose columns in (s8, b)
            # order and E tiles for one timestep are contiguous 16-col runs
            bmul = small.tile([P, 1], i32, tag="bmul")
            nc.vector.tensor_scalar(bmul[:], bq[:], 3, None,
                                    op0=Alu.logical_shift_left)
            s8i = small.tile([P, 1], i32, tag="s8i")
            nc.vector.tensor_sub(s8i[:], piota[:], bmul[:])
            permv = small.tile([P, 1], i32, tag="permv")
            nc.vector.tensor_scalar(permv[:], s8i[:], 4, None,
                                    op0=Alu.logical_shift_left)
            nc.vector.tensor_add(permv[:], permv[:], bq[:])
            permvf = small.tile([P, 1], f32, tag="permvf")
            nc.vector.tensor_copy(permvf[:], permv[:])
            iotap = small.tile([P, P], i32, tag="iotap")
            nc.gpsimd.iota(iotap[:], pattern=[[1, P]], base=0,
                           channel_multiplier=0)
            iotapf = small.tile([P, P], f32, tag="iotapf")
            nc.vector.tensor_copy(iotapf[:], iotap[:])
            perm = small.tile([P, P], bf16, tag="perm")
            nc.vector.tensor_scalar(perm[:], iotapf[:], permvf[:], None,
                                    op0=Alu.is_equal)

            # ---------------- weights ----------------
            W = small.tile([JA, JA], f32, tag="W")
            nc.vector.memset(W[:], 0.0)
            nc.sync.dma_start(W[:J, :J], T_d.ap())
            nc.sync.dma_start(W[:J, J : J + 1], en_d.ap())
            nc.scalar.activation(W[:J, :], W[:J, :], Exp)
            nc.vector.memset(W[J : J + 1, J : J + 1], 1.0)
            # WT = transpose(W) so that matmul(., WT, x) = W @ x
            ptw = ptrans.tile([JA, JA], f32, tag="pt")
            nc.tensor.matmul(ptw[:], W[:], idn33[:], is_transpose=True,
                             start=True, stop=True)
            WT = small.tile([JA, JA], f32, tag="WT")
            nc.scalar.copy(WT[:], ptw[:])

            est = small.tile([J, 1], f32, tag="est")
            nc.sync.dma_start(est[:], st_d.ap())
            nc.scalar.activation(est[:], est[:], Exp)
            enp = small.tile([JA, 1], f32, tag="enp")
            nc.sync.dma_start(enp[:J, :], en_d.ap())
            nc.scalar.activation(enp[:J, :], enp[:J, :], Exp)
            nc.vector.memset(enp[J : J + 1, :], 1.0)
            ones_ja = small.tile([JA, 1], f32, tag="ones_ja")
            nc.vector.memset(ones_ja[:], 1.0)
            ones_p = small.tile([P, 1], f32, tag="ones_p")
            nc.vector.memset(ones_p[:], 1.0)
            ones_b = small.tile([B, 1], f32, tag="ones_b")
            nc.vector.memset(ones_b[:], 1.0)

            # ---------------- emission prep, [128, .] layout ----------------
            # emraw[p, f*J + j] = em[b, t, j], p = b*8 + s8, t = s8*64 + f
            emraw = big.tile([P, FS * J], f32, tag="emraw")
            expP = big.tile([P, FS * J], f32, tag="expP")
            ssum = small.tile([P, FS], f32, tag="ssum")
            s_eff = small.tile([P, FS], f32, tag="s_eff")
            rs128 = small.tile([P, FS], f32, tag="rs128")
            c_log = small.tile([P, FS], f32, tag="c_log")
            # emA: exp-normalized emissions (bf16) augmented with the omega
            # multiplier as tag 32, padded to 64 cols per timestep so the
            # transposed E tiles land at partition offsets 0/64 (32-aligned)
            emA = big.tile([P, FS * KPAD], bf16, tag="emA")
            emA3 = emA[:].rearrange("p (f k) -> p f k", k=KPAD)
            nc.gpsimd.memset(emA[:], 0.0)
            emraw3 = emraw[:].rearrange("p (f j) -> p f j", j=J)
            expP3 = expP[:].rearrange("p (f j) -> p f j", j=J)
            em_flat = em_d.ap().rearrange("b s j -> (b s j)").rearrange(
                "(p f) -> p f", p=P
            )

            # process in f-quarters so the scan chains can start early;
            # order 0,3,1,2 so both chain heads (t=1 needs f=0..2, t=511
            # needs f=63) are fed first.
            FQ = FS // 4
            for q in (0, 3, 1, 2):
                fsl = slice(q * FQ, (q + 1) * FQ)
                csl = slice(q * FQ * J, (q + 1) * FQ * J)
                nc.sync.dma_start(emraw[:, csl], em_flat[:, csl])
                nc.scalar.activation(expP[:, csl], emraw[:, csl], Exp)
                nc.vector.tensor_reduce(
                    ssum[:, fsl], expP3[:, fsl, :], axis=Ax.X, op=Alu.add
                )
                # s_eff = (ssum - 1) * mask + 1   (=1 on padded steps)
                nc.vector.scalar_tensor_tensor(
                    s_eff[:, fsl], ssum[:, fsl], -1.0, mask128[:, fsl],
                    op0=Alu.add, op1=Alu.mult,
                )
                nc.vector.tensor_scalar(
                    s_eff[:, fsl], s_eff[:, fsl], 1.0, None, op0=Alu.add
                )
                # rs = mask / s_eff ; c_log = ln(s_eff)
                nc.vector.reciprocal(rs128[:, fsl], s_eff[:, fsl])
                nc.vector.tensor_mul(rs128[:, fsl], rs128[:, fsl],
                                     mask128[:, fsl])
                nc.scalar.activation(c_log[:, fsl], s_eff[:, fsl], Ln)
                # normalized exp into the augmented slab (bf16)
                nc.vector.tensor_tensor(
                    emA3[:, fsl, :J],
                    expP3[:, fsl, :],
                    rs128[:, fsl].unsqueeze(2).broadcast_to([P, FQ, J]),
                    op=Alu.mult,
                )
                # omega multiplier column: 1 - mask
                nc.vector.tensor_scalar(
                    emA3[:, fsl, J], mask128[:, fsl], -1.0, 1.0,
                    op0=Alu.mult, op1=Alu.add,
                )

            # csum total = sum over (p, f) of c_log
            csred = small.tile([P, 1], f32, tag="csred")
            nc.vector.tensor_reduce(csred[:], c_log[:], axis=Ax.X, op=Alu.add)

            # ---------------- transpose to scan layout ----------------
            # escan[sl*64 + j, ck, s8*16 + b] = emA[b*8+s8, (2*ck+sl)*64+j]
            # via plain matmul emA_slice^T @ perm (transpose + column perm)
            escan = big.tile([P, NCK * P], f32, tag="escan")
            escan3 = escan[:].rearrange("r (c p) -> r c p", p=P)
            # ping-pong order so both chain heads are fed immediately
            cks = []
            lo, hi = 0, NCK - 1
            while lo <= hi:
                cks.append(hi)
                if lo != hi:
                    cks.append(lo)
                hi -= 1
                lo += 1
            for ck in cks:
                pt = ptrans.tile([P, P], f32, tag="pt")
                nc.tensor.matmul(
                    pt[:], emA[:, ck * KPAD * TPC : (ck + 1) * KPAD * TPC],
                    perm[:], start=True, stop=True
                )
                nc.scalar.copy(escan3[:, ck, :], pt[:])

            def e_ap(t):
                s8, f = t // FS, t % FS
                ck, sl = f // TPC, f % TPC
                return escan3[sl * KPAD : sl * KPAD + JA, ck,
                              s8 * B : (s8 + 1) * B]

            # ---------------- score setup (one-hots on GPSIMD) -----------
            woven = []
            if with_score:
                len128m1 = small.tile([P, 1], f32, tag="len128m1")
                nc.vector.tensor_scalar(len128m1[:], len128[:], -1.0, None,
                                        op0=Alu.add)
                tgt128 = small.tile([P, FS], i32, tag="tgt128")
                tgt_flat = tgt_d.ap().rearrange("b s -> (b s)")
                tgtv = tgt_flat.rearrange("(p f) -> p f", p=P)
                nc.sync.dma_start(tgt128[:], tgtv)
                tgt128f = small.tile([P, FS], f32, tag="tgt128f")
                nc.gpsimd.tensor_copy(tgt128f[:], tgt128[:])
                # shifted targets: tgt[b, t+1] at (p, f); garbage at the
                # seam is always masked (the t=511 pair is never valid)
                tgtn128 = small.tile([P, FS], i32, tag="tgtn128")
                nc.vector.memset(tgtn128[:, FS - 1 : FS], 0)
                nc.sync.dma_start(tgtn128[:, : FS - 1], tgtv[:, 1:])
                nc.sync.dma_start(tgtn128[: P - 1, FS - 1 : FS], tgtv[1:, 0:1])
                tgtn128f = small.tile([P, FS], f32, tag="tgtn128f")
                nc.gpsimd.tensor_copy(tgtn128f[:], tgtn128[:])
                maskn128 = small.tile([P, FS], f32, tag="maskn128")
                nc.vector.tensor_scalar(maskn128[:], tv128[:], len128m1[:],
                                        None, op0=Alu.is_lt)
                last128 = small.tile([P, FS], f32, tag="last128")
                nc.vector.tensor_scalar(last128[:], tv128[:], len128m1[:],
                                        None, op0=Alu.is_equal)
                # masked target codes: tgt where valid else -1
                tgtmP = small.tile([P, FS], f32, tag="tgtmP")
                nc.gpsimd.tensor_scalar(tgtmP[:], tgt128f[:], 1.0, None,
                                        op0=Alu.add)
                nc.gpsimd.tensor_mul(tgtmP[:], tgtmP[:], mask128[:])
                nc.gpsimd.tensor_scalar(tgtmP[:], tgtmP[:], -1.0, None,
                                        op0=Alu.add)
                tgtmN = small.tile([P, FS], f32, tag="tgtmN")
                nc.gpsimd.tensor_scalar(tgtmN[:], tgtn128f[:], 1.0, None,
                                        op0=Alu.add)
                nc.gpsimd.tensor_mul(tgtmN[:], tgtmN[:], maskn128[:])
                nc.gpsimd.tensor_scalar(tgtmN[:], tgtmN[:], -1.0, None,
                                        op0=Alu.add)
                # one-hot slabs [P, FS, J]
                iota_ji = small.tile([P, J], i32, tag="iota_ji")
                nc.gpsimd.iota(iota_ji[:], pattern=[[1, J]], base=0,
                               channel_multiplier=0)
                iota_jf = small.tile([P, J], f32, tag="iota_jf")
                nc.gpsimd.tensor_copy(iota_jf[:], iota_ji[:])
                iota_b = iota_jf[:].unsqueeze(1).broadcast_to([P, FS, J])
                ohp = big.tile([P, FS * J], f32, tag="ohp")
                ohp3 = ohp[:].rearrange("p (f j) -> p f j", j=J)
                nc.vector.tensor_tensor(
                    ohp3, iota_b,
                    tgtmP[:].unsqueeze(2).broadcast_to([P, FS, J]),
                    op=Alu.is_equal,
                )
                ohn = big.tile([P, FS * J], f32, tag="ohn")
                ohn3 = ohn[:].rearrange("p (f j) -> p f j", j=J)
                nc.vector.tensor_tensor(
                    ohn3, iota_b,
                    tgtmN[:].unsqueeze(2).broadcast_to([P, FS, J]),
                    op=Alu.is_equal,
                )
                # three PSUM-accumulating matmul groups, woven between the
                # scan matmuls (PE is ~30% busy during the chains):
                #   cpsum[i,j]  += sum_f ohp_f^T ohn_f   (transition counts)
                #   empsum[i,j] += sum_f ohp_f^T emraw_f (diag = emission)
                #   endpsum[j]  += sum_f ohp_f^T last_f  (end-tag counts)
                cpe = pfin.tile([J, J + 1], f32, tag="fin_b")
                cpsum = cpe[:, :J]
                endpsum = cpe[:, J : J + 1]
                empsum = pfin.tile([J, J], f32, tag="fin_c")
                for f in range(FS):
                    woven.append((cpsum, ohp3[:, f, :], ohn3[:, f, :], f))
                    woven.append((empsum[:], ohp3[:, f, :],
                                  emraw3[:, f, :], f))
                    woven.append((endpsum, ohp3[:, f, :],
                                  last128[:, f : f + 1], f))

            # ---------------- the two scan chains ----------------
            # forward: A_t = (W^T A_{t-1}) .* E_t,  t = 1..HALF
            a_prev = afp.tile([JA, B], f32, tag="a")
            nc.vector.tensor_scalar(
                a_prev[:J, :], e_ap(0)[:J, :], est[:], None, op0=Alu.mult
            )
            nc.vector.memset(a_prev[J : J + 1, :], 0.0)
            # backward: B_{t-1} = W (E_t .* B_t),  t = S-1 .. HALF+1
            # B_init lives in PSUM (diag(enp) @ ones) so every backward
            # multiply is PSUM x SBUF — SB/SB ops would need equal base
            # partitions, which the 0/64-alternating E tiles violate.
            enpdiag = small.tile([JA, JA], f32, tag="enpdiag")
            nc.vector.tensor_scalar(enpdiag[:], idn33[:], enp[:], None,
                                    op0=Alu.mult)
            ones_jb = small.tile([JA, B], f32, tag="ones_jb")
            nc.vector.memset(ones_jb[:], 1.0)
            b_prev = psb.tile([JA, B], f32, tag="pb")
            nc.tensor.matmul(b_prev[:], enpdiag[:], ones_jb[:], start=True,
                             stop=True)

            wi = 0
            for r in range(rounds):  # 255 paired rounds
                tf, tb = r + 1, S - 1 - r
                # forward step tf
                ps = psf.tile([JA, B], f32, tag="ps")
                nc.tensor.matmul(ps[:], W[:], a_prev[:], start=True, stop=True)
                a_t = afp.tile([JA, B], f32, tag="a")
                nc.vector.tensor_mul(a_t[:], ps[:], e_ap(tf))
                a_prev = a_t
                # backward step tb (b_prev stays in PSUM between rounds;
                # the next round's multiply drains it)
                m = mbp.tile([JA, B], f32, tag="m")
                nc.vector.tensor_mul(m[:], b_prev[:], e_ap(tb))
                pb = psb.tile([JA, B], f32, tag="pb")
                nc.tensor.matmul(pb[:], WT[:], m[:], start=True, stop=True)
                b_prev = pb
                # weave one score matmul per round (delayed so the one-hot
                # slabs are ready before the first one enters PE's queue)
                if r >= WSTART and wi < len(woven):
                    out, lhsT, rhs, f = woven[wi]
                    wi += 1
                    nc.tensor.matmul(out, lhsT, rhs, start=(f == 0),
                                     stop=(f == FS - 1),
                                     skip_group_check=True)
            # one extra forward step to reach t = HALF
            ps = psf.tile([JA, B], f32, tag="ps")
            nc.tensor.matmul(ps[:], W[:], a_prev[:], start=True, stop=True)
            a_t = afp.tile([JA, B], f32, tag="a")
            nc.vector.tensor_mul(a_t[:], ps[:], e_ap(HALF))
            a_prev = a_t
            # drain any not-yet-woven score matmuls (short-rounds builds)
            while wi < len(woven):
                out, lhsT, rhs, f = woven[wi]
                wi += 1
                nc.tensor.matmul(out, lhsT, rhs, start=(f == 0),
                                 stop=(f == FS - 1), skip_group_check=True)

            # ---------------- combine: Z_b = sum_j A[j] B[j] ----------------
            v = small.tile([JA, B], f32, tag="v")
            nc.vector.tensor_mul(v[:], a_prev[:], b_prev[:])
            zrow = pfin.tile([1, B], f32, tag="fin_a")
            nc.tensor.matmul(zrow[:], ones_ja[:], v[:], start=True, stop=True)
            logz = small.tile([1, B], f32, tag="logz")
            nc.scalar.activation(logz[:], zrow[:], Ln)

            if with_score:
                # transition term: sum_ij cpsum * T
                Traw = small.tile([J, J], f32, tag="Traw")
                nc.sync.dma_start(Traw[:], T_d.ap())
                tsc = small.tile([J, 1], f32, tag="tsc")
                tscratch = small.tile([J, J], f32, tag="tscratch")
                nc.vector.tensor_mul(tscratch[:], cpsum, Traw[:])
                nc.vector.tensor_reduce(tsc[:], tscratch[:], axis=Ax.X,
                                        op=Alu.add)
                # emission term: diag(empsum)
                emdiag = small.tile([J, 1], f32, tag="emdiag")
                emscr = small.tile([J, J], f32, tag="emscr")
                nc.vector.tensor_mul(emscr[:], empsum[:], idn33[:J, :J])
                nc.vector.tensor_reduce(emdiag[:], emscr[:], axis=Ax.X,
                                        op=Alu.add)
                # end term
                en_raw = small.tile([J, 1], f32, tag="en_raw")
                nc.sync.dma_start(en_raw[:], en_d.ap())
                endsc = small.tile([J, 1], f32, tag="endsc")
                nc.vector.tensor_mul(endsc[:], endpsum, en_raw[:])
                # start term: counts of tgt[b, 0]
                tgt0 = small.tile([B, 1], i32, tag="tgt0")
                nc.sync.dma_start(tgt0[:], tgt_d.ap()[:, 0:1])
                tgt0f = small.tile([B, 1], f32, tag="tgt0f")
                nc.vector.tensor_copy(tgt0f[:], tgt0[:])
                iota_jb = small.tile([B, J], i32, tag="iota_jb")
                nc.gpsimd.iota(iota_jb[:], pattern=[[1, J]], base=0,
                               channel_multiplier=0)
                iota_jbf = small.tile([B, J], f32, tag="iota_jbf")
                nc.vector.tensor_copy(iota_jbf[:], iota_jb[:])
                oh0 = small.tile([B, J], f32, tag="oh0")
                nc.vector.tensor_scalar(oh0[:], iota_jbf[:], tgt0f[:], None,
                                        op0=Alu.is_equal)
                cnt0 = pfin.tile([J, 1], f32, tag="fin_a")
                nc.tensor.matmul(cnt0[:], oh0[:], ones_b[:], start=True,
                                 stop=True)
                st_raw = small.tile([J, 1], f32, tag="st_raw")
                nc.sync.dma_start(st_raw[:], st_d.ap())
                stsc = small.tile([J, 1], f32, tag="stsc")
                nc.vector.tensor_mul(stsc[:], cnt0[:], st_raw[:])

            # ---------------- combine ----------------
            # NLL = sum_b logz + sum(c_log) - sum_j(tsc+emdiag+endsc+stsc)
            s_all = pfin.tile([1, 1], f32, tag="fin_a")
            nc.tensor.matmul(s_all[:], csred[:], ones_p[:], start=True,
                             stop=True)
            s2 = small.tile([1, 1], f32, tag="s2")
            nc.vector.tensor_reduce(s2[:], logz[:], axis=Ax.X, op=Alu.add)
            res = small.tile([1, 1], f32, tag="res")
            nc.vector.tensor_add(res[:], s_all[:], s2[:])
            if with_score:
                sneg = small.tile([J, 1], f32, tag="sneg")
                nc.vector.tensor_add(sneg[:], tsc[:], endsc[:])
                nc.vector.tensor_add(sneg[:], sneg[:], stsc[:])
                nc.vector.tensor_add(sneg[:], sneg[:], emdiag[:])
                nc.vector.tensor_scalar(sneg[:], sneg[:], -1.0, None,
                                        op0=Alu.mult)
                ones_j = small.tile([J, 1], f32, tag="ones_j")
                nc.vector.memset(ones_j[:], 1.0)
                s3p = pfin.tile([1, 1], f32, tag="fin_a")
                nc.tensor.matmul(s3p[:], ones_j[:], sneg[:], start=True,
                                 stop=True)
                nc.vector.tensor_add(res[:], res[:], s3p[:])
            nc.sync.dma_start(out_d.ap(), res[:])

    nc.compile()
    return nc


_NC_CACHE = None


def kernel(emission, length, target, transition, start_transition,
           end_transition):
    global _NC_CACHE
    from concourse.bass_utils import run_bass_kernel_spmd

    emission = np.ascontiguousarray(np.asarray(emission, np.float32))
    length = np.asarray(length).astype(np.int32).reshape(-1, 1)
    target = np.asarray(target).astype(np.int32)
    transition = np.ascontiguousarray(np.asarray(transition, np.float32))
    start = np.asarray(start_transition, np.float32).reshape(J, 1)
    end = np.asarray(end_transition, np.float32).reshape(J, 1)

    if _NC_CACHE is None:
        _NC_CACHE = build_bass()
    nc = _NC_CACHE

    in_maps = []
    for c in range(NCORES):
        sl = slice(c * B, (c + 1) * B)
        in_maps.append({
            "emission": np.ascontiguousarray(emission[sl]),
            "length": np.ascontiguousarray(length[sl]),
            "target": np.ascontiguousarray(target[sl]),
            "transition": transition,
            "start_transition": start,
            "end_transition": end,
        })

    r = run_bass_kernel_spmd(nc, in_maps, list(range(NCORES)))
    total = np.float64(0.0)
    for c in range(NCORES):
        total += np.float64(r.results[c]["out"][0, 0])
    return np.asarray(total, np.float32)


if __name__ == "__main__":
    rng = np.random.default_rng(0)
    inputs = {
        "emission": rng.standard_normal((128, S, J)).astype(np.float32),
        "length": rng.integers(2, S + 1, size=(128,)),
        "target": rng.integers(0, J, size=(128, S)),
        "transition": (rng.standard_normal((J, J)) * 0.1).astype(np.float32),
        "start_transition": (rng.standard_normal(J) * 0.1).astype(np.float32),
        "end_transition": (rng.standard_normal(J) * 0.1).astype(np.float32),
    }
    print(kernel(**inputs))
